# revision 42
# baseline (speedup 1.0000x reference)
"""Trainium2 Bass kernel for nn_CustomConv2D: gather 16x16 patches at given
centers and apply a shared [768 -> 1024] linear projection + bias.

Sharding: data-parallel over batch across 8 NeuronCores (8 images/core,
4608 patches/core); weight+bias replicated.

Host prepares im2col patches in k-major bf16 layout (contraction on
partitions); the device runs a pure accumulating-matmul pipeline.
bf16 operands run the PE at the same 1 cycle/row as fp32r but halve HBM
traffic (the fp32 version is DMA-co-bound at ~36MB/core); measured rel
err vs the fp32 reference is ~4e-3. Patch extraction runs on host:
TRN2's SWDGE indirect-DMA costs ~1.4us/instruction (measured), so any
device-side gather of 221k patch rows is ~2.4ms -- off the roofline.

Perf structure (from NTFF traces; fixed NEFF preamble+epilogue is
~17us, PE floor is 432 matmuls x 216ns = 93.3us):
- TWO HALF-PASSES over the output columns: pass A computes
  out[:, :512] for all 36 blocks, pass B computes out[:, 512:].
  Pass A only needs the h0 half of the weights (768KB) before full-rate
  streaming, halving the 8-core HBM burst at kernel start that
  otherwise stalls the first blocks (per-core effective HBM is only
  ~150-250GB/s while all 8 cores load weights+patches at once).
- gt chunks persist in SBUF (55KB/partition, fits alongside weights),
  so pass B has zero input-DMA dependency and cannot stall.
- Early DMAs are interleaved across the Sync and Scalar HWDGE rings in
  consumption order (GpSimd's SWDGE ring is ~1.4us/instr -- bias only).
- Dummy 512-row matmuls on a zeroed tile pre-ramp the PE pstate
  (0.65 -> 1.2 -> 2.4GHz over ~3us of continuous busy) during the
  initial DMA wait; an idle gap resets the ramp, so the count is sized
  to end just as the first weights+patches land.
- Output is staged [P, NBLK*O] on device (contiguous-run stores per
  chunk-half) and transposed back on host.

Set CONV_MM_DT=f32r / f32 for higher-precision fallbacks.
"""

import os
import numpy as np
import ml_dtypes

import concourse.bass as bass
from concourse import bacc
import concourse.mybir as mybir
import concourse.tile as tile

# problem shape (hardcoded per contract)
B, C, H, W = 64, 3, 384, 384
N, K, O = 576, 16, 1024
NCORES = 8
B_LOC = B // NCORES          # 8 images per core
NPC = B_LOC * N              # 4608 patches per core
P = 128                      # partitions / patches per block
NBLK = NPC // P              # 36 blocks
KDIM = C * K * K             # 768 contraction dim
KSL = KDIM // P              # 6 k-slices
HO = O // 2                  # half of the output columns (one pass)

CBS = [1, 1, 2, 2, 2] + [4] * 6 + [2, 1, 1]   # blocks per gt chunk
assert sum(CBS) == NBLK
GTLEN = KSL * P * NBLK       # flat gt columns per partition

# Dummy 512-row matmuls bridging from engine-preamble end (~7.1us) to the
# MEASURED data-ready point (~12.4us: all h0 weights + gt0 landed on both
# rings; per-instruction evt_wait_time in the NTFF trace, not bandwidth
# math -- early contended HBM is ~half the naive estimate). Any idle gap
# between warmups and the real stream resets the PE pstate ramp, so bridge
# the full window: 7 strides at MID pace (~427ns) complete the ramp, the
# remaining 10 run at ~216ns. At N=17 the handoff waits vanish on 7/8
# cores; more would tax every core ~0.2us per extra warmup.
N_WARMUP = 17

MM_DT = os.environ.get("CONV_MM_DT", "bf16")


def _build(reps: int = 1):
    nc = bacc.Bacc()
    f32 = mybir.dt.float32
    mm_dt = {"f32": f32, "f32r": mybir.dt.float32r,
             "bf16": mybir.dt.bfloat16}[MM_DT]
    out_dt = f32 if MM_DT in ("f32", "f32r") else mybir.dt.bfloat16

    gt_t = nc.declare_dram_parameter("gt", [P, GTLEN], mm_dt, isOutput=False)
    wt_t = nc.declare_dram_parameter("wt", [P, KSL, O], mm_dt, isOutput=False)
    bias_t = nc.declare_dram_parameter("bias", [P, O], mm_dt, isOutput=False)
    out_t = nc.declare_dram_parameter("out", [P, NBLK * O], out_dt,
                                      isOutput=True)

    with tile.TileContext(nc) as tc:
        with (
            tc.tile_pool(name="const", bufs=1) as cpool,
            tc.tile_pool(name="osb", bufs=3) as opool,
            tc.tile_pool(name="outp", bufs=7, space="PSUM") as psumpool,
            tc.tile_pool(name="warm", bufs=1, space="PSUM") as wpsum,
        ):
            # PE warm-up: zeroed operands, result never read. Nothing in the
            # kernel touches GpSimd: an unused SWDGE ring makes its (expensive)
            # drain in the fixed NEFF epilogue trivial.
            z_sb = cpool.tile([P, 640], mm_dt)
            nc.vector.memset(z_sb[:], 0.0)
            zps = wpsum.tile([P, 512], f32)
            for _ in range(N_WARMUP):
                nc.tensor.matmul(zps[:], lhsT=z_sb[:, :128],
                                 rhs=z_sb[:, 128:640], start=True, stop=True)

            # gt chunks are persistent SBUF tiles (pass B reuses them)
            gt_sb = [cpool.tile([P, KSL * P * cb], mm_dt, tag=f"gtc{ci}",
                                name=f"gtc{ci}")
                     for ci, cb in enumerate(CBS)]
            wt_sb = cpool.tile([P, KSL, O], mm_dt)

            # Stream start is gated on ALL h0 weights + gt0 (0.97MB): balance
            # exactly that set across the two HWDGE rings ahead of everything
            # else (gt0 itself split across rings), then the rest in
            # consumption order.
            off = [0]
            gt_offs = []
            for ci, cb in enumerate(CBS):
                gt_offs.append(off[0])
                off[0] += KSL * P * cb

            def load_gt(ci, eng):
                L = KSL * P * CBS[ci]
                eng.dma_start(gt_sb[ci][:], gt_t[:, gt_offs[ci]:gt_offs[ci] + L])

            nc.sync.dma_start(wt_sb[:, 0, :HO], wt_t[:, 0, :HO])
            load_gt(0, nc.scalar)
            nc.sync.dma_start(wt_sb[:, 1, :HO], wt_t[:, 1, :HO])
            nc.scalar.dma_start(wt_sb[:, 3, :HO], wt_t[:, 3, :HO])
            nc.sync.dma_start(wt_sb[:, 2, :HO], wt_t[:, 2, :HO])
            nc.scalar.dma_start(wt_sb[:, 4, :HO], wt_t[:, 4, :HO])
            nc.scalar.dma_start(wt_sb[:, 5, :HO], wt_t[:, 5, :HO])
            # bias comes host-replicated [P, O] bf16 (256KB): h0 half early on
            # sync (needed by the first DVE add ~13us), h1 half + h1 weights
            # behind the early pieces (pass B starts ~55us in)
            bias_bc = cpool.tile([P, O], mm_dt)
            nc.sync.dma_start(bias_bc[:, :HO], bias_t[:, :HO])
            load_gt(1, nc.scalar)
            load_gt(2, nc.sync)
            load_gt(3, nc.scalar)
            load_gt(4, nc.sync)
            for ci in range(5, len(CBS)):
                load_gt(ci, nc.sync)

            for ks in range(KSL):
                nc.scalar.dma_start(wt_sb[:, ks, HO:], wt_t[:, ks, HO:])
            nc.scalar.dma_start(bias_bc[:, HO:], bias_t[:, HO:])

            def half_pass(h):
                hs = slice(h * HO, (h + 1) * HO)
                blk = 0
                for ci, cb in enumerate(CBS):
                    o_sb = opool.tile([P, cb, HO], out_dt, tag=f"o{cb}_{h}",
                                      name=f"o{cb}_{h}")
                    for b in range(cb):
                        out_ps = psumpool.tile([P, HO], f32, tag="outp")
                        for ks in range(KSL):
                            nc.tensor.matmul(
                                out_ps[:],
                                lhsT=gt_sb[ci][:, ks * cb * P + b * P:
                                               ks * cb * P + (b + 1) * P],
                                rhs=wt_sb[:, ks, hs],
                                start=(ks == 0), stop=(ks == KSL - 1),
                            )
                        nc.vector.tensor_add(o_sb[:, b, :], out_ps[:],
                                             bias_bc[:, hs])
                    # store [P, cb, HO] -> out[P, blk..blk+cb, h-half]
                    dest = (out_t[:, blk * O:(blk + cb) * O]
                            .rearrange("p (c o) -> p c o", c=cb)[:, :, hs])
                    nc.scalar.dma_start(dest, o_sb[:])
                    blk += cb

            def body(_i=None):
                for h in range(2):
                    half_pass(h)

            if reps == 1:
                body()
            else:
                with tc.For_i(0, reps, 1) as i:
                    body(i)
    nc.finalize()
    return nc


_CACHE = {}


def _get_nc(reps: int = 1):
    if reps not in _CACHE:
        _CACHE[reps] = _build(reps)
    return _CACHE[reps]


def _np_dt():
    return {"f32": np.float32, "f32r": np.float32,
            "bf16": ml_dtypes.bfloat16}[MM_DT]


def _prep_inputs(x, centers, weight, bias):
    x = np.ascontiguousarray(x, dtype=np.float32)
    centers = np.asarray(centers, dtype=np.int64)
    weight = np.ascontiguousarray(weight, dtype=np.float32)
    bias = np.ascontiguousarray(bias, dtype=np.float32)
    np_dt = _np_dt()

    # host im2col: patches [B, N, C*K*K]
    win = np.lib.stride_tricks.sliding_window_view(x, (K, K), axis=(2, 3))
    r0 = centers[:, :, 0] - K // 2        # [B, N]
    c0 = centers[:, :, 1] - K // 2
    b_ids = np.arange(B)[:, None]
    patches = win[b_ids, :, r0, c0]       # [B, N, C, K, K]

    # weight [O, C, K, K] -> wT [KDIM, O] -> [128, KSL, O]
    wflat = weight.reshape(O, KDIM)
    wt_host = np.ascontiguousarray(
        wflat.T.reshape(KSL, P, O).transpose(1, 0, 2)).astype(np_dt)
    bias_host = np.ascontiguousarray(
        np.broadcast_to(bias.reshape(1, O), (P, O))).astype(np_dt)

    in_maps = []
    for core in range(NCORES):
        pc = patches[core * B_LOC:(core + 1) * B_LOC].reshape(NPC, KDIM)
        pcT = np.ascontiguousarray(pc.T).astype(np_dt)  # [KDIM, NPC]
        # chunk-contiguous flat layout: chunk = [P, KSL, cb*P] at gt_off
        gt_host = np.empty((P, GTLEN), dtype=np_dt)
        off = 0
        blk = 0
        for cb in CBS:
            L = KSL * P * cb
            # [KDIM, cb*P] -> [KSL, P, cb*P] -> [P, KSL*cb*P]
            chunk = pcT[:, blk * P:(blk + cb) * P].reshape(KSL, P, cb * P)
            gt_host[:, off:off + L] = (
                chunk.transpose(1, 0, 2).reshape(P, L))
            off += L
            blk += cb
        in_maps.append({"gt": gt_host, "wt": wt_host, "bias": bias_host})
    return in_maps


def kernel(x, centers, weight, bias):
    from concourse.bass_utils import run_bass_kernel_spmd
    nc = _get_nc(1)
    in_maps = _prep_inputs(x, centers, weight, bias)
    res = run_bass_kernel_spmd(nc, in_maps, list(range(NCORES))).results
    # device out: [P, NBLK*O] (row p, block t at t*O) -> [NPC, O]
    outs = []
    for i in range(NCORES):
        o = np.asarray(res[i]["out"]).astype(np.float32)
        outs.append(o.reshape(P, NBLK, O).transpose(1, 0, 2))
    out = np.stack(outs, axis=0)
    return np.ascontiguousarray(out.reshape(B, N, O))


# revision 44
# speedup vs baseline: 1.0024x; 1.0024x over previous
"""Trainium2 Bass kernel for nn_CustomConv2D: gather 16x16 patches at given
centers and apply a shared [768 -> 1024] linear projection + bias.

Sharding: data-parallel over batch across 8 NeuronCores (8 images/core,
4608 patches/core); weight+bias replicated.

Host prepares im2col patches in k-major bf16 layout (contraction on
partitions); the device runs a pure accumulating-matmul pipeline.
bf16 operands run the PE at the same 1 cycle/row as fp32r but halve HBM
traffic (the fp32 version is DMA-co-bound at ~36MB/core); measured rel
err vs the fp32 reference is ~4e-3. Patch extraction runs on host:
TRN2's SWDGE indirect-DMA costs ~1.4us/instruction (measured), so any
device-side gather of 221k patch rows is ~2.4ms -- off the roofline.

Perf structure (from NTFF traces; fixed NEFF preamble+epilogue is
~17us, PE floor is 432 matmuls x 216ns = 93.3us):
- TWO HALF-PASSES over the output columns: pass A computes
  out[:, :512] for all 36 blocks, pass B computes out[:, 512:].
  Pass A only needs the h0 half of the weights (768KB) before full-rate
  streaming, halving the 8-core HBM burst at kernel start that
  otherwise stalls the first blocks (per-core effective HBM is only
  ~150-250GB/s while all 8 cores load weights+patches at once).
- gt chunks persist in SBUF (55KB/partition, fits alongside weights),
  so pass B has zero input-DMA dependency and cannot stall.
- Early DMAs are interleaved across the Sync and Scalar HWDGE rings in
  consumption order (GpSimd's SWDGE ring is ~1.4us/instr -- bias only).
- Dummy 512-row matmuls on a zeroed tile pre-ramp the PE pstate
  (0.65 -> 1.2 -> 2.4GHz over ~3us of continuous busy) during the
  initial DMA wait; an idle gap resets the ramp, so the count is sized
  to end just as the first weights+patches land.
- Output is staged [P, NBLK*O] on device (contiguous-run stores per
  chunk-half) and transposed back on host.

Set CONV_MM_DT=f32r / f32 for higher-precision fallbacks.
"""

import os
import numpy as np
import ml_dtypes

import concourse.bass as bass
from concourse import bacc
import concourse.mybir as mybir
import concourse.tile as tile

# problem shape (hardcoded per contract)
B, C, H, W = 64, 3, 384, 384
N, K, O = 576, 16, 1024
NCORES = 8
B_LOC = B // NCORES          # 8 images per core
NPC = B_LOC * N              # 4608 patches per core
P = 128                      # partitions / patches per block
NBLK = NPC // P              # 36 blocks
KDIM = C * K * K             # 768 contraction dim
KSL = KDIM // P              # 6 k-slices
HO = O // 2                  # half of the output columns (one pass)

CBS = [1, 1, 2, 2, 2] + [4] * 6 + [2, 1, 1]   # blocks per gt chunk
assert sum(CBS) == NBLK
GTLEN = KSL * P * NBLK       # flat gt columns per partition

# Dummy 512-row matmuls bridging from engine-preamble end (~7.1us) to the
# MEASURED data-ready point (~12.4us: all h0 weights + gt0 landed on both
# rings; per-instruction evt_wait_time in the NTFF trace, not bandwidth
# math -- early contended HBM is ~half the naive estimate). Any idle gap
# between warmups and the real stream resets the PE pstate ramp, so bridge
# the full window: 7 strides at MID pace (~427ns) complete the ramp, the
# remaining 10 run at ~216ns. At N=17 the handoff waits vanish on 7/8
# cores; more would tax every core ~0.2us per extra warmup.
N_WARMUP = 17

MM_DT = os.environ.get("CONV_MM_DT", "bf16")


def _build(reps: int = 1):
    nc = bacc.Bacc()
    f32 = mybir.dt.float32
    mm_dt = {"f32": f32, "f32r": mybir.dt.float32r,
             "bf16": mybir.dt.bfloat16}[MM_DT]
    out_dt = f32 if MM_DT in ("f32", "f32r") else mybir.dt.bfloat16

    gt_t = nc.declare_dram_parameter("gt", [P, GTLEN], mm_dt, isOutput=False)
    wt_t = nc.declare_dram_parameter("wt", [P, KSL, O], mm_dt, isOutput=False)
    bias_t = nc.declare_dram_parameter("bias", [P, O], mm_dt, isOutput=False)
    out_t = nc.declare_dram_parameter("out", [P, NBLK * O], out_dt,
                                      isOutput=True)

    with tile.TileContext(nc) as tc:
        with (
            tc.tile_pool(name="const", bufs=1) as cpool,
            tc.tile_pool(name="osb", bufs=3) as opool,
            tc.tile_pool(name="outp", bufs=7, space="PSUM") as psumpool,
            tc.tile_pool(name="warm", bufs=1, space="PSUM") as wpsum,
        ):
            # PE warm-up: zeroed operands, result never read. Nothing in the
            # kernel touches GpSimd: an unused SWDGE ring makes its (expensive)
            # drain in the fixed NEFF epilogue trivial.
            z_sb = cpool.tile([P, 640], mm_dt)
            nc.vector.memset(z_sb[:], 0.0)
            zps = wpsum.tile([P, 512], f32)
            for _ in range(N_WARMUP):
                nc.tensor.matmul(zps[:], lhsT=z_sb[:, :128],
                                 rhs=z_sb[:, 128:640], start=True, stop=True)

            # gt chunks are persistent SBUF tiles (pass B reuses them)
            gt_sb = [cpool.tile([P, KSL * P * cb], mm_dt, tag=f"gtc{ci}",
                                name=f"gtc{ci}")
                     for ci, cb in enumerate(CBS)]
            wt_sb = cpool.tile([P, KSL, O], mm_dt)

            # Stream start is gated on ALL h0 weights + gt0 (0.97MB): balance
            # exactly that set across the two HWDGE rings ahead of everything
            # else (gt0 itself split across rings), then the rest in
            # consumption order.
            off = [0]
            gt_offs = []
            for ci, cb in enumerate(CBS):
                gt_offs.append(off[0])
                off[0] += KSL * P * cb

            def load_gt(ci, eng):
                L = KSL * P * CBS[ci]
                eng.dma_start(gt_sb[ci][:], gt_t[:, gt_offs[ci]:gt_offs[ci] + L])

            nc.sync.dma_start(wt_sb[:, 0, :HO], wt_t[:, 0, :HO])
            load_gt(0, nc.scalar)
            nc.sync.dma_start(wt_sb[:, 1, :HO], wt_t[:, 1, :HO])
            nc.scalar.dma_start(wt_sb[:, 3, :HO], wt_t[:, 3, :HO])
            nc.sync.dma_start(wt_sb[:, 2, :HO], wt_t[:, 2, :HO])
            nc.scalar.dma_start(wt_sb[:, 4, :HO], wt_t[:, 4, :HO])
            nc.scalar.dma_start(wt_sb[:, 5, :HO], wt_t[:, 5, :HO])
            # bias comes host-replicated [P, O] bf16 (256KB): h0 half early on
            # sync (needed by the first DVE add ~13us), h1 half + h1 weights
            # behind the early pieces (pass B starts ~55us in)
            bias_bc = cpool.tile([P, O], mm_dt)
            nc.sync.dma_start(bias_bc[:, :HO], bias_t[:, :HO])
            load_gt(1, nc.scalar)
            load_gt(2, nc.sync)
            load_gt(3, nc.scalar)
            load_gt(4, nc.sync)
            for ci in range(5, len(CBS)):
                load_gt(ci, nc.sync)

            for ks in range(KSL):
                nc.scalar.dma_start(wt_sb[:, ks, HO:], wt_t[:, ks, HO:])
            nc.scalar.dma_start(bias_bc[:, HO:], bias_t[:, HO:])

            def half_pass(h):
                hs = slice(h * HO, (h + 1) * HO)
                blk = 0
                for ci, cb in enumerate(CBS):
                    o_sb = opool.tile([P, cb, HO], out_dt, tag=f"o{cb}_{h}",
                                      name=f"o{cb}_{h}")
                    for b in range(cb):
                        out_ps = psumpool.tile([P, HO], f32, tag="outp")
                        for ks in range(KSL):
                            nc.tensor.matmul(
                                out_ps[:],
                                lhsT=gt_sb[ci][:, ks * cb * P + b * P:
                                               ks * cb * P + (b + 1) * P],
                                rhs=wt_sb[:, ks, hs],
                                start=(ks == 0), stop=(ks == KSL - 1),
                            )
                        nc.vector.tensor_add(o_sb[:, b, :], out_ps[:],
                                             bias_bc[:, hs])
                    # store [P, cb, HO] -> out[P, blk..blk+cb, h-half]
                    dest = (out_t[:, blk * O:(blk + cb) * O]
                            .rearrange("p (c o) -> p c o", c=cb)[:, :, hs])
                    nc.scalar.dma_start(dest, o_sb[:])
                    blk += cb

            def body(_i=None):
                for h in range(2):
                    half_pass(h)

            if reps == 1:
                body()
            else:
                with tc.For_i(0, reps, 1) as i:
                    body(i)
    nc.finalize()
    return nc


_CACHE = {}


def _get_nc(reps: int = 1):
    if reps not in _CACHE:
        _CACHE[reps] = _build(reps)
    return _CACHE[reps]


def _np_dt():
    return {"f32": np.float32, "f32r": np.float32,
            "bf16": ml_dtypes.bfloat16}[MM_DT]


def _prep_inputs(x, centers, weight, bias):
    x = np.ascontiguousarray(x, dtype=np.float32)
    centers = np.asarray(centers, dtype=np.int64)
    weight = np.ascontiguousarray(weight, dtype=np.float32)
    bias = np.ascontiguousarray(bias, dtype=np.float32)
    np_dt = _np_dt()

    # host im2col: patches [B, N, C*K*K]
    win = np.lib.stride_tricks.sliding_window_view(x, (K, K), axis=(2, 3))
    r0 = centers[:, :, 0] - K // 2        # [B, N]
    c0 = centers[:, :, 1] - K // 2
    b_ids = np.arange(B)[:, None]
    patches = win[b_ids, :, r0, c0]       # [B, N, C, K, K]

    # weight [O, C, K, K] -> wT [KDIM, O] -> [128, KSL, O]
    wflat = weight.reshape(O, KDIM)
    wt_host = np.ascontiguousarray(
        wflat.T.reshape(KSL, P, O).transpose(1, 0, 2)).astype(np_dt)
    bias_host = np.ascontiguousarray(
        np.broadcast_to(bias.reshape(1, O), (P, O))).astype(np_dt)

    in_maps = []
    for core in range(NCORES):
        pc = patches[core * B_LOC:(core + 1) * B_LOC].reshape(NPC, KDIM)
        pcT = np.ascontiguousarray(pc.T).astype(np_dt)  # [KDIM, NPC]
        # chunk-contiguous flat layout: chunk = [P, KSL, cb*P] at gt_off
        gt_host = np.empty((P, GTLEN), dtype=np_dt)
        off = 0
        blk = 0
        for cb in CBS:
            L = KSL * P * cb
            # [KDIM, cb*P] -> [KSL, P, cb*P] -> [P, KSL*cb*P]
            chunk = pcT[:, blk * P:(blk + cb) * P].reshape(KSL, P, cb * P)
            gt_host[:, off:off + L] = (
                chunk.transpose(1, 0, 2).reshape(P, L))
            off += L
            blk += cb
        in_maps.append({"gt": gt_host, "wt": wt_host, "bias": bias_host})
    return in_maps


def kernel(x, centers, weight, bias):
    from concourse.bass_utils import run_bass_kernel_spmd
    nc = _get_nc(1)
    in_maps = _prep_inputs(x, centers, weight, bias)
    res = run_bass_kernel_spmd(nc, in_maps, list(range(NCORES))).results
    # device out: [P, NBLK*O] (row p, block t at t*O) -> [NPC, O]
    outs = []
    for i in range(NCORES):
        o = np.asarray(res[i]["out"]).astype(np.float32)
        outs.append(o.reshape(P, NBLK, O).transpose(1, 0, 2))
    out = np.stack(outs, axis=0)
    return np.ascontiguousarray(out.reshape(B, N, O))
